# revision 5
# baseline (speedup 1.0000x reference)
"""Trainium2 Bass kernel for AttentionWithRelPos (v2).

Data-parallel over batch B=16 across 8 NeuronCores (2 batches/core).
Attention computed in "S^T" orientation (keys on partitions, queries on
the free dim).  All matmuls run in bf16 (1 cycle/row on the PE):

  - qkv projection emits Q^T/K^T in [d, token] layout (bf16) and V in
    [token, head, d+1] layout with a ones column for the softmax
    denominator.
  - Keys processed in 5 chunks [CLS+120, 120, 120, 120, 96]; per chunk
    the PE accumulates QK^T, the rel-pos bias (identity matmul over a
    precomputed block-Toeplitz table slice) and the additive mask into
    PSUM; the scalar engine exponentiates straight out of PSUM (no max
    subtraction -- logits are provably tiny, masked entries underflow
    to 0); PV accumulates in a second PSUM tile.
  - Softmax denominators are stacked head-wise on 32-aligned partitions
    so ONE vector-engine reciprocal covers heads 0-3 (and one more
    heads 4-5) -- reciprocal cost is per-instruction, not per-partition.
  - Division happens on the 64x580 head output (tensor_tensor multiply
    with a gpsimd partition-broadcast reciprocal), output in bf16 feeds
    the output projection directly.
  - Big batched DMAs (one per logical tensor) spread across the sync /
    scalar / gpsimd queues so issue cost never serializes compute.
"""

import sys

if '/opt/trn_rl_repo' not in sys.path:
    sys.path.insert(0, '/opt/trn_rl_repo')

import numpy as np
import ml_dtypes

import concourse.bass as bass
import concourse.mybir as mybir
from concourse import bacc
from concourse.tile import TileContext
from concourse.masks import make_identity
from concourse import bass_utils

B, N, C, H = 16, 577, 384, 6
NQ = 580                    # padded query axis (2 banks x 290)
HEAD_DIM = C // H           # 64
SCALE = HEAD_DIM ** -0.5
NB = 2                      # batches per core
NCORES = 8
F32 = mybir.dt.float32
BF16 = mybir.dt.bfloat16
MSBW = 1060                 # padded msb table width (1056 + 4)

# key chunks in token space: (token0, rows). Chunk 0 includes CLS.
CHUNKS = [(0, 121), (121, 120), (241, 120), (361, 120), (481, 96)]
QW = 290                    # query columns per psum bank
TSL = [(0, 128), (128, 128), (256, 128), (384, 128), (512, 65)]


def _mm(nc, out, lhsT, rhs, **kw):
    nc.tensor.matmul(out, lhsT, rhs, skip_group_check=True, **kw)


def build_program(patch_attn: bool, nobias: bool = False):
    nc = bacc.Bacc("TRN2", target_bir_lowering=False, debug=False,
                   enable_asserts=False)

    xT_d = nc.dram_tensor("xT", [NB, 3, 128, NQ], BF16, kind="ExternalInput")
    cmb_d = nc.dram_tensor("cmb", [NB, H, 121, 5, NQ], BF16,
                           kind="ExternalInput")
    wqkvqk_d = nc.dram_tensor("wqkvqk", [3, 128, 2 * C], BF16,
                              kind="ExternalInput")
    wqkvv_d = nc.dram_tensor("wqkvv", [3, 128, C], BF16,
                             kind="ExternalInput")
    wproj_d = nc.dram_tensor("wproj", [128, 3, C], BF16,
                             kind="ExternalInput")
    # cblob: [qk bias (6) | v bias (384) | proj bias (384)]
    cb_d = nc.dram_tensor("cblob", [128, 6 + 2 * C], F32,
                          kind="ExternalInput")
    out_d = nc.dram_tensor("out", [NB, N, C], BF16, kind="ExternalOutput")

    with TileContext(nc) as tc:
        with (
            tc.tile_pool(name="const", bufs=1) as cpool,
            tc.tile_pool(name="batch", bufs=2) as bpool,
            tc.tile_pool(name="ptile", bufs=3) as ppool,
            tc.tile_pool(name="small", bufs=1) as spool,
            tc.tile_pool(name="psum", bufs=2, space="PSUM") as pspool,
        ):
            # ---------------- input DMAs (big, one per tensor) ----------
            # DMA rings are FIFO per issuing engine: order by first use.
            # sync ring:   cb, wqkv-QK, wqkv-V, msb, mask b0, mask b1
            # scalar ring: x b0, x b1, wproj
            wqkv = cpool.tile([128, 3, 3 * C], BF16, tag="wqkv")
            cb = cpool.tile([128, 6 + 2 * C], F32, tag="cb")
            if not nobias:
                nc.sync.dma_start(cb[:], cb_d[:])
            for ci in range(3):
                nc.sync.dma_start(wqkv[:, ci, :2 * C], wqkvqk_d[ci])
            xts = []
            for b in range(NB):
                t = bpool.tile([128, 3, NQ], BF16, tag="xt", name=f"xt{b}")
                for ci in range(3):
                    nc.scalar.dma_start(t[:, ci, :], xT_d[b, ci])
                xts.append(t)
            for ci in range(3):
                nc.sync.dma_start(wqkv[:, ci, 2 * C:], wqkvv_d[ci])
            # combined bias+mask tables stream per (batch, head) on the sync
            # ring; bufs=3 gives a natural prefetch depth of three heads
            cmbs = {}
            for b in range(NB):
                for h in range(H):
                    t = bpool.tile([121, 5, NQ], BF16, tag="cmb", bufs=3,
                                   name=f"cmb{b}_{h}")
                    nc.sync.dma_start(t[:], cmb_d[b, h])
                    cmbs[(b, h)] = t
            wproj = cpool.tile([128, 3, C], BF16, tag="wproj")
            nc.scalar.dma_start(wproj[:], wproj_d[:])

            # ---------------- constants ----------------
            ident = cpool.tile([128, 128], BF16, tag="ident")
            make_identity(nc, ident[:])
            # HAM warmup: ~30 dummy matmuls with no DMA dependency keep
            # the PE busy (and ramp its clock) while inputs stream in
            wps = pspool.tile([128, 2, 512], F32, tag="sp", name="warm")
            for i in range(35):
                _mm(nc, wps[:, i % 2, :128], ident[:, :], ident[:, :128],
                    start=True, stop=True)

            def proj(b):
                xt = xts[b]
                qkt = []
                for oi in range(6):
                    ps = pspool.tile([128, 2, 512], F32, tag="sp")
                    for bk in range(2):
                        for ci in range(3):
                            _mm(nc, ps[:, bk, :QW],
                                wqkv[:, ci, 128 * oi:128 * (oi + 1)],
                                xt[:, ci, QW * bk:QW * (bk + 1)],
                                start=(ci == 0), stop=(ci == 2))
                    if oi < 3:
                        # per-head zero-padded Q tiles: head data on the rows
                        # matching its K-tile rows, zeros elsewhere, so the
                        # QK matmul runs full-array K=128 (h64 row-group
                        # matmuls stall the LDW pipeline)
                        for sub in range(2):
                            h = 2 * oi + sub
                            po = 64 * sub
                            t = bpool.tile([128, 2, QW], BF16, tag=f"qh{h}",
                                           name=f"qh{h}_{b}")
                            nc.gpsimd.memset(t[64 - po:128 - po, :, :], 0.0)
                            if nobias:
                                nc.vector.tensor_copy(
                                    t[po:po + 64, :, :],
                                    ps[po:po + 64, :, :QW])
                            else:
                                nc.vector.tensor_scalar_add(
                                    t[po:po + 64, :, :],
                                    ps[po:po + 64, :, :QW],
                                    cb[po:po + 64, oi:oi + 1])
                            qkt.append(t)
                    else:
                        t = bpool.tile([128, 2, QW], BF16, tag=f"qkt{oi}",
                                       name=f"qkt{oi}_{b}")
                        if nobias:
                            nc.scalar.copy(t[:, :, :], ps[:, :, :QW])
                        else:
                            nc.vector.tensor_scalar_add(
                                t[:, :, :], ps[:, :, :QW], cb[:, oi:oi + 1])
                        qkt.append(t)
                vts = []
                for c, (t0, rows) in enumerate(CHUNKS):
                    ps = pspool.tile([128, 2, 512], F32, tag="sp")
                    for ci in range(3):
                        _mm(nc, ps[:rows, 0, :C], xt[:, ci, t0:t0 + rows],
                            wqkv[:, ci, 2 * C:3 * C],
                            start=(ci == 0), stop=(ci == 2))
                    t = bpool.tile([121, H, HEAD_DIM + 1], BF16,
                                   tag=f"vt{c}", name=f"vt{c}_{b}")
                    if nobias:
                        nc.scalar.copy(
                            t[:rows, :, :HEAD_DIM],
                            ps[:rows, 0, :C].rearrange("p (h d) -> p h d",
                                                       h=H))
                    else:
                        nc.vector.tensor_tensor(
                            t[:rows, :, :HEAD_DIM],
                            ps[:rows, 0, :C].rearrange("p (h d) -> p h d",
                                                       h=H),
                            cb[:rows, 6:6 + C].rearrange("p (h d) -> p h d",
                                                         h=H),
                            mybir.AluOpType.add)
                    nc.gpsimd.memset(t[:rows, :, HEAD_DIM:HEAD_DIM + 1], 1.0)
                    vts.append(t)
                return qkt, vts

            def attn_init(b, qkt, vts):
                # denominator stacks: heads 0-3 at partitions 0/32/64/96,
                # head 4 (and head 5 for batch 0) on a second tile
                st = dict(b=b, qkt=qkt, vts=vts, och=[])
                st['stA'] = spool.tile([97, 2, QW], F32, tag="stA", bufs=2,
                                       name=f"stA{b}")
                st['stB'] = spool.tile([33, 2, QW], F32, tag="stB", bufs=2,
                                       name=f"stB{b}")
                nc.gpsimd.memset(st['stA'][:], 1.0)
                nc.gpsimd.memset(st['stB'][:], 1.0)
                st['recA'] = spool.tile([97, 2, QW], BF16, tag="recA",
                                        bufs=2, name=f"recA{b}")
                st['recB'] = spool.tile([33, 2, QW], BF16, tag="recB",
                                        bufs=2, name=f"recB{b}")
                st['x2t'] = [bpool.tile([128, 2, QW], BF16, tag=f"x2t{ci}",
                                        name=f"x2t{ci}_{b}")
                             for ci in range(3)]
                return st

            def div_prep(st, hh, rc, rr):
                # hoist the p32k->p0 shift copies ahead of the broadcast
                # chain so the gpsimd broadcasts queue back-to-back
                if rr == 0:
                    return rc
                rt = spool.tile([1, 2, QW], BF16, tag="rtmp",
                                bufs=3, name=f"rt{hh}_{st['b']}")
                nc.vector.tensor_copy(rt[:, :, :], rc[rr:rr + 1, :, :])
                return rt

            def division(st, hh, rc, rr=0):
                b, x2t, och = st['b'], st['x2t'], st['och']
                tih, poh = hh // 2, 64 * (hh % 2)
                recb = spool.tile([64, 2, QW], BF16, tag="recb",
                                  bufs=4, name=f"recb{hh}_{b}")
                nc.gpsimd.partition_broadcast(recb[:, :, :],
                                              rc[rr:rr + 1, :, :])
                nc.vector.tensor_tensor(
                    x2t[tih][poh:poh + 64, :, :].rearrange("p a b -> p (a b)"),
                    och[hh][:, :, :].rearrange("p a b -> p (a b)"),
                    recb[:, :, :].rearrange("p a b -> p (a b)"),
                    mybir.AluOpType.mult)

            def attn_head(st, h, last_direct):
                b, qkt, vts, och = st['b'], st['qkt'], st['vts'], st['och']
                cmbt = cmbs[(b, h)]
                ti, po = h // 2, 64 * (h % 2)
                qT = qkt[h]          # zero-padded per-head Q
                kT = qkt[6 + ti]     # packed K pair (other head x 0)
                if True:
                    ov = pspool.tile([65, 2, 512], F32, tag="ov")
                    for c, (t0, rows) in enumerate(CHUNKS):
                        sp = pspool.tile([128, 2, 512], F32, tag="sp")
                        for bk in range(2):
                            _mm(nc, sp[:rows, bk, :QW],
                                kT[:, :, :].rearrange(
                                    "p a b -> p (a b)")[:, t0:t0 + rows],
                                qT[:, bk, :],
                                start=True, stop=False)
                            _mm(nc, sp[:rows, bk, :QW],
                                ident[:rows, :rows],
                                cmbt[:rows, c, QW * bk:QW * (bk + 1)],
                                start=False, stop=True)
                        pt = ppool.tile([128, 2, QW], BF16, tag="pt")
                        nc.scalar.activation(
                            pt[:rows, :, :], sp[:rows, :, :QW],
                            mybir.ActivationFunctionType.Exp)
                        for bk in range(2):
                            _mm(nc, ov[:, bk, :QW], vts[c][:rows, h, :],
                                pt[:rows, bk, :],
                                start=(c == 0), stop=(c == 4))
                    # stack the denominator first (the reciprocal waits
                    # on it), then copy the head output out of PSUM
                    stA, stB = st['stA'], st['stB']
                    recA, recB = st['recA'], st['recB']
                    if h >= 4:
                        nc.vector.tensor_copy(
                            stB[32 * (h - 4):32 * (h - 4) + 1, :, :],
                            ov[64:65, :, :QW])
                    oc = spool.tile([64, 2, QW], BF16, tag="och", bufs=4,
                                    name=f"och{h}_{b}")
                    nc.vector.tensor_copy(oc[:, :, :], ov[:64, :, :QW])
                    och.append(oc)
                    if h < 4:
                        nc.vector.tensor_copy(stA[32 * h:32 * h + 1, :, :],
                                              ov[64:65, :, :QW])
                        if h == 3:
                            with nc.allow_low_precision(reason="softmax den"):
                                nc.vector.reciprocal(recA[:, :, :],
                                                     stA[:, :, :])
                            rcs = [div_prep(st, hh, recA, 32 * hh)
                                   for hh in range(4)]
                            for hh in range(4):
                                division(st, hh, rcs[hh])
                    elif h == 5 and last_direct:
                        # final head of the kernel: reciprocal straight from
                        # PSUM (no stacking) to shorten the tail chain
                        rc5 = spool.tile([1, 2, QW], BF16, tag="rc5")
                        with nc.allow_low_precision(reason="softmax den"):
                            nc.vector.reciprocal(rc5[:, :, :],
                                                 ov[64:65, :, :QW])
                        division(st, 5, rc5, 0)
                    else:
                        if h == 4 and last_direct:
                            with nc.allow_low_precision(reason="softmax den"):
                                nc.vector.reciprocal(recB[:33, :, :],
                                                     stB[:33, :, :])
                            division(st, 4, recB, 0)
                        elif h == 5:
                            with nc.allow_low_precision(reason="softmax den"):
                                nc.vector.reciprocal(recB[:, :, :],
                                                     stB[:, :, :])
                            rc5 = div_prep(st, 5, recB, 32)
                            division(st, 4, recB)
                            division(st, 5, rc5)

            def outproj(b, x2t):
                for si, (t0, tn) in enumerate(TSL):
                    ps = pspool.tile([128, 2, 512], F32, tag="sp")
                    for ci in range(3):
                        _mm(nc, ps[:tn, 0, :C],
                            x2t[ci][:, :, :].rearrange(
                                "p a b -> p (a b)")[:, t0:t0 + tn],
                            wproj[:, ci, :], start=(ci == 0), stop=(ci == 2))
                    yt = spool.tile([128, C], BF16, tag="yt", bufs=3,
                                    name=f"yt{si}_{b}")
                    if nobias:
                        nc.scalar.copy(yt[:tn, :], ps[:tn, 0, :C])
                    else:
                        nc.vector.tensor_tensor(yt[:tn, :], ps[:tn, 0, :C],
                                                cb[:tn, 6 + C:6 + 2 * C],
                                                mybir.AluOpType.add)
                    (nc.sync if si % 2 == 0 else nc.gpsimd).dma_start(
                        out_d[b, t0:t0 + tn, :], yt[:tn, :])

            qkt0, vts0 = proj(0)
            qkt1, vts1 = proj(1)
            st0 = attn_init(0, qkt0, vts0)
            st1 = attn_init(1, qkt1, vts1)
            for h in range(H):
                attn_head(st0, h, last_direct=False)
            for h in range(H):
                attn_head(st1, h, last_direct=False)
            outproj(0, st0['x2t'])
            outproj(1, st1['x2t'])

    nc.compile()
    return nc


def prep_inputs(x, qkv_w, qkv_b, proj_w, proj_b, rel_pos, rel_pos_index,
                mask, patch_attn):
    x = np.asarray(x, dtype=np.float32)
    qkv_w = np.asarray(qkv_w, dtype=np.float32)
    qkv_b = np.asarray(qkv_b, dtype=np.float32)
    proj_w = np.asarray(proj_w, dtype=np.float32)
    proj_b = np.asarray(proj_b, dtype=np.float32)
    rel_pos = np.asarray(rel_pos, dtype=np.float32)
    mask = np.asarray(mask)

    # x^T padded to 580 query columns, [B, 3, 128, 580] -> [B,128,3,580]
    xT = np.zeros((B, C, NQ), dtype=np.float32)
    xT[:, :, :N] = x.transpose(0, 2, 1)
    xT = np.ascontiguousarray(xT.reshape(B, 3, 128, NQ)).astype(
        ml_dtypes.bfloat16)

    W = qkv_w.copy()
    W[:C] *= np.float32(SCALE)
    b2 = qkv_b.copy()
    b2[:C] *= np.float32(SCALE)
    # wqkv: [C, 3C]^T packed [128, 3, 3C]
    wT3 = W.T.reshape(3, 128, 3 * C)
    wqkvqk = np.ascontiguousarray(wT3[:, :, :2 * C]).astype(ml_dtypes.bfloat16)
    wqkvv = np.ascontiguousarray(wT3[:, :, 2 * C:]).astype(ml_dtypes.bfloat16)
    wproj = np.ascontiguousarray(
        proj_w.T.reshape(3, 128, C).transpose(1, 0, 2)
    ).astype(ml_dtypes.bfloat16)
    cblob = np.zeros((128, 6 + 2 * C), dtype=np.float32)
    cblob[:, :6] = b2[:2 * C].reshape(6, 128).T
    cblob[:, 6:6 + C] = np.broadcast_to(b2[2 * C:], (128, C))
    cblob[:, 6 + C:] = np.broadcast_to(proj_b, (128, C))

    # combined additive table: mask (0 / -1e30, per batch) + rel-pos bias
    # (per head), in [key, chunk, query] layout
    mb = np.zeros((B, N + 1, NQ), dtype=np.float32)
    mb[:, :N, :N] = ((mask.transpose(0, 2, 1).astype(np.float32) - 1.0)
                     * np.float32(1e30))
    if patch_attn:
        # biasT[h, key, query] over full token indices (0 = CLS, no bias)
        biasT = np.zeros((H, N, NQ), dtype=np.float32)
        rpi = np.asarray(rel_pos_index)
        biasT[:, 1:N, 1:N] = rel_pos[:, rpi].transpose(0, 2, 1)
    cmb = np.zeros((B, H, 121, 5, NQ), dtype=ml_dtypes.bfloat16)
    for b in range(B):
        for c, (t0, rows) in enumerate(CHUNKS):
            blk = mb[b, None, t0:t0 + rows, :]
            if patch_attn:
                blk = blk + biasT[:, t0:t0 + rows, :]
            cmb[b, :, :rows, c, :] = blk.astype(ml_dtypes.bfloat16)

    shared = {
        "wqkvqk": wqkvqk, "wqkvv": wqkvv, "wproj": wproj,
        "cblob": cblob,
    }
    in_maps = []
    for i in range(NCORES):
        m = dict(shared)
        m["xT"] = np.ascontiguousarray(xT[NB * i:NB * (i + 1)])
        m["cmb"] = np.ascontiguousarray(cmb[NB * i:NB * (i + 1)])
        in_maps.append(m)
    return in_maps


_NC_CACHE = {}


def _get_nc(patch_attn: bool, nobias: bool):
    key = (bool(patch_attn), bool(nobias))
    if key not in _NC_CACHE:
        _NC_CACHE[key] = build_program(*key)
    return _NC_CACHE[key]


def kernel(**inputs):
    patch_attn = bool(np.asarray(inputs["patch_attn"]))
    nobias = (not np.any(np.asarray(inputs["qkv_b"]))
              and not np.any(np.asarray(inputs["proj_b"])))
    nc = _get_nc(patch_attn, nobias)
    in_maps = prep_inputs(**inputs)
    res = bass_utils.run_bass_kernel_spmd(nc, in_maps,
                                          core_ids=list(range(NCORES)))
    out = np.concatenate([res.results[i]["out"] for i in range(NCORES)],
                         axis=0)
    return np.ascontiguousarray(out.astype(np.float32))


# revision 6
# speedup vs baseline: 1.1672x; 1.1672x over previous
"""Trainium2 Bass kernel for AttentionWithRelPos.

Data-parallel over batch B=16 across 8 NeuronCores (2 batches/core).
Attention computed in "S^T" orientation (keys on partitions, queries on
the free dim).  All matmuls run in bf16 (1 cycle/row on the PE):

  - qkv projection emits per-head zero-padded Q tiles (so QK matmuls run
    full-array K=128: K=64 row-group matmuls stall the LDWEIGHTS
    pipeline ~2x), packed K tiles, and V in [token, head, d+1] layout
    with a ones column that yields the softmax denominator for free.
  - Keys processed in 5 chunks [CLS+120, 120, 120, 120, 96]; per chunk
    the PE accumulates QK^T plus ONE host-precomputed combined
    (rel-pos bias + additive mask) table matmul into PSUM; the scalar
    engine exponentiates straight out of PSUM (no max subtraction --
    logits are provably tiny; masked entries hold -1e30 and underflow
    to 0).  The combined table is exact for arbitrary rel_pos_index.
  - Softmax denominators are stacked head-wise onto 32-aligned
    partitions so one vector-engine reciprocal covers heads 0-3 and one
    covers 4-5 per batch (reciprocal costs ~3.8us per instruction,
    independent of partition count); shift copies are hoisted ahead of
    the gpsimd partition-broadcasts so those queue back-to-back.
  - Division happens on the 64x580 head output (tensor_tensor multiply
    against the broadcast reciprocal), writing bf16 x2t tiles that feed
    the output projection directly; outputs store as bf16.
  - DMA rings are FIFO per issuing engine: loads are contiguous slabs
    ordered by first use across the sync/scalar rings, with ~35 HAM
    warmup matmuls (no DMA dependency) covering the load window.

Scheduling invariants learned the hard way: never add allocations to
the sp/ov PSUM rotation (statically-scheduled queues entangle, +10-27us
each attempt) and never put latency-critical work on the scalar queue
behind the exp stream (strict FIFO).
"""

import sys

if '/opt/trn_rl_repo' not in sys.path:
    sys.path.insert(0, '/opt/trn_rl_repo')

import numpy as np
import ml_dtypes

import concourse.bass as bass
import concourse.mybir as mybir
from concourse import bacc
from concourse.tile import TileContext
from concourse.masks import make_identity
from concourse import bass_utils

B, N, C, H = 16, 577, 384, 6
NQ = 580                    # padded query axis (2 banks x 290)
HEAD_DIM = C // H           # 64
SCALE = HEAD_DIM ** -0.5
NB = 2                      # batches per core
NCORES = 8
F32 = mybir.dt.float32
BF16 = mybir.dt.bfloat16
MSBW = 1060                 # padded msb table width (1056 + 4)

# key chunks in token space: (token0, rows). Chunk 0 includes CLS.
CHUNKS = [(0, 121), (121, 120), (241, 120), (361, 120), (481, 96)]
QW = 290                    # query columns per psum bank
TSL = [(0, 128), (128, 128), (256, 128), (384, 128), (512, 65)]


def _mm(nc, out, lhsT, rhs, **kw):
    nc.tensor.matmul(out, lhsT, rhs, skip_group_check=True, **kw)


def build_program(patch_attn: bool, nobias: bool = False):
    nc = bacc.Bacc("TRN2", target_bir_lowering=False, debug=False,
                   enable_asserts=False)

    xT_d = nc.dram_tensor("xT", [NB, 3, 128, NQ], BF16, kind="ExternalInput")
    cmb_d = nc.dram_tensor("cmb", [NB, H, 121, 5, NQ], BF16,
                           kind="ExternalInput")
    wqkvqk_d = nc.dram_tensor("wqkvqk", [3, 128, 2 * C], BF16,
                              kind="ExternalInput")
    wqkvv_d = nc.dram_tensor("wqkvv", [3, 128, C], BF16,
                             kind="ExternalInput")
    wproj_d = nc.dram_tensor("wproj", [128, 3, C], BF16,
                             kind="ExternalInput")
    # cblob: [qk bias (6) | v bias (384) | proj bias (384)]
    cb_d = nc.dram_tensor("cblob", [128, 6 + 2 * C], F32,
                          kind="ExternalInput")
    out_d = nc.dram_tensor("out", [NB, N, C], BF16, kind="ExternalOutput")

    with TileContext(nc) as tc:
        with (
            tc.tile_pool(name="const", bufs=1) as cpool,
            tc.tile_pool(name="batch", bufs=2) as bpool,
            tc.tile_pool(name="ptile", bufs=3) as ppool,
            tc.tile_pool(name="small", bufs=1) as spool,
            tc.tile_pool(name="psum", bufs=2, space="PSUM") as pspool,
        ):
            # ---------------- input DMAs (big, one per tensor) ----------
            # DMA rings are FIFO per issuing engine: order by first use.
            # sync ring:   cb, wqkv-QK, wqkv-V, msb, mask b0, mask b1
            # scalar ring: x b0, x b1, wproj
            wqkv = cpool.tile([128, 3, 3 * C], BF16, tag="wqkv")
            cb = cpool.tile([128, 6 + 2 * C], F32, tag="cb")
            if not nobias:
                nc.sync.dma_start(cb[:], cb_d[:])
            for ci in range(3):
                nc.sync.dma_start(wqkv[:, ci, :2 * C], wqkvqk_d[ci])
            xts = []
            for b in range(NB):
                t = bpool.tile([128, 3, NQ], BF16, tag="xt", name=f"xt{b}")
                for ci in range(3):
                    nc.scalar.dma_start(t[:, ci, :], xT_d[b, ci])
                xts.append(t)
            for ci in range(3):
                nc.sync.dma_start(wqkv[:, ci, 2 * C:], wqkvv_d[ci])
            # combined bias+mask tables stream per (batch, head) on the sync
            # ring; bufs=3 gives a natural prefetch depth of three heads
            cmbs = {}
            for b in range(NB):
                for h in range(H):
                    t = bpool.tile([121, 5, NQ], BF16, tag="cmb", bufs=3,
                                   name=f"cmb{b}_{h}")
                    nc.sync.dma_start(t[:], cmb_d[b, h])
                    cmbs[(b, h)] = t
            wproj = cpool.tile([128, 3, C], BF16, tag="wproj")
            nc.scalar.dma_start(wproj[:], wproj_d[:])

            # ---------------- constants ----------------
            ident = cpool.tile([128, 128], BF16, tag="ident")
            make_identity(nc, ident[:])
            # HAM warmup: ~30 dummy matmuls with no DMA dependency keep
            # the PE busy (and ramp its clock) while inputs stream in
            wps = pspool.tile([128, 2, 512], F32, tag="sp", name="warm")
            for i in range(35):
                _mm(nc, wps[:, i % 2, :128], ident[:, :], ident[:, :128],
                    start=True, stop=True)

            def proj(b):
                xt = xts[b]
                qkt = []
                for oi in range(6):
                    ps = pspool.tile([128, 2, 512], F32, tag="sp")
                    for bk in range(2):
                        for ci in range(3):
                            _mm(nc, ps[:, bk, :QW],
                                wqkv[:, ci, 128 * oi:128 * (oi + 1)],
                                xt[:, ci, QW * bk:QW * (bk + 1)],
                                start=(ci == 0), stop=(ci == 2))
                    if oi < 3:
                        # per-head zero-padded Q tiles: head data on the rows
                        # matching its K-tile rows, zeros elsewhere, so the
                        # QK matmul runs full-array K=128 (h64 row-group
                        # matmuls stall the LDW pipeline)
                        for sub in range(2):
                            h = 2 * oi + sub
                            po = 64 * sub
                            t = bpool.tile([128, 2, QW], BF16, tag=f"qh{h}",
                                           name=f"qh{h}_{b}")
                            nc.gpsimd.memset(t[64 - po:128 - po, :, :], 0.0)
                            if nobias:
                                nc.vector.tensor_copy(
                                    t[po:po + 64, :, :],
                                    ps[po:po + 64, :, :QW])
                            else:
                                nc.vector.tensor_scalar_add(
                                    t[po:po + 64, :, :],
                                    ps[po:po + 64, :, :QW],
                                    cb[po:po + 64, oi:oi + 1])
                            qkt.append(t)
                    else:
                        t = bpool.tile([128, 2, QW], BF16, tag=f"qkt{oi}",
                                       name=f"qkt{oi}_{b}")
                        if nobias:
                            nc.scalar.copy(t[:, :, :], ps[:, :, :QW])
                        else:
                            nc.vector.tensor_scalar_add(
                                t[:, :, :], ps[:, :, :QW], cb[:, oi:oi + 1])
                        qkt.append(t)
                vts = []
                for c, (t0, rows) in enumerate(CHUNKS):
                    ps = pspool.tile([128, 2, 512], F32, tag="sp")
                    for ci in range(3):
                        _mm(nc, ps[:rows, 0, :C], xt[:, ci, t0:t0 + rows],
                            wqkv[:, ci, 2 * C:3 * C],
                            start=(ci == 0), stop=(ci == 2))
                    t = bpool.tile([121, H, HEAD_DIM + 1], BF16,
                                   tag=f"vt{c}", name=f"vt{c}_{b}")
                    if nobias:
                        nc.scalar.copy(
                            t[:rows, :, :HEAD_DIM],
                            ps[:rows, 0, :C].rearrange("p (h d) -> p h d",
                                                       h=H))
                    else:
                        nc.vector.tensor_tensor(
                            t[:rows, :, :HEAD_DIM],
                            ps[:rows, 0, :C].rearrange("p (h d) -> p h d",
                                                       h=H),
                            cb[:rows, 6:6 + C].rearrange("p (h d) -> p h d",
                                                         h=H),
                            mybir.AluOpType.add)
                    nc.gpsimd.memset(t[:rows, :, HEAD_DIM:HEAD_DIM + 1], 1.0)
                    vts.append(t)
                return qkt, vts

            def attn_init(b, qkt, vts):
                # denominator stacks: heads 0-3 at partitions 0/32/64/96,
                # head 4 (and head 5 for batch 0) on a second tile
                st = dict(b=b, qkt=qkt, vts=vts, och=[])
                st['stA'] = spool.tile([97, 2, QW], F32, tag="stA", bufs=2,
                                       name=f"stA{b}")
                st['stB'] = spool.tile([33, 2, QW], F32, tag="stB", bufs=2,
                                       name=f"stB{b}")
                nc.gpsimd.memset(st['stA'][:], 1.0)
                nc.gpsimd.memset(st['stB'][:], 1.0)
                st['recA'] = spool.tile([97, 2, QW], BF16, tag="recA",
                                        bufs=2, name=f"recA{b}")
                st['recB'] = spool.tile([33, 2, QW], BF16, tag="recB",
                                        bufs=2, name=f"recB{b}")
                st['x2t'] = [bpool.tile([128, 2, QW], BF16, tag=f"x2t{ci}",
                                        name=f"x2t{ci}_{b}")
                             for ci in range(3)]
                return st

            def div_prep(st, hh, rc, rr):
                # hoist the p32k->p0 shift copies ahead of the broadcast
                # chain so the gpsimd broadcasts queue back-to-back
                if rr == 0:
                    return rc
                rt = spool.tile([1, 2, QW], BF16, tag="rtmp",
                                bufs=3, name=f"rt{hh}_{st['b']}")
                nc.vector.tensor_copy(rt[:, :, :], rc[rr:rr + 1, :, :])
                return rt

            def division(st, hh, rc, rr=0):
                b, x2t, och = st['b'], st['x2t'], st['och']
                tih, poh = hh // 2, 64 * (hh % 2)
                recb = spool.tile([64, 2, QW], BF16, tag="recb",
                                  bufs=4, name=f"recb{hh}_{b}")
                nc.gpsimd.partition_broadcast(recb[:, :, :],
                                              rc[rr:rr + 1, :, :])
                nc.vector.tensor_tensor(
                    x2t[tih][poh:poh + 64, :, :].rearrange("p a b -> p (a b)"),
                    och[hh][:, :, :].rearrange("p a b -> p (a b)"),
                    recb[:, :, :].rearrange("p a b -> p (a b)"),
                    mybir.AluOpType.mult)

            def attn_head(st, h, last_direct):
                b, qkt, vts, och = st['b'], st['qkt'], st['vts'], st['och']
                cmbt = cmbs[(b, h)]
                ti, po = h // 2, 64 * (h % 2)
                qT = qkt[h]          # zero-padded per-head Q
                kT = qkt[6 + ti]     # packed K pair (other head x 0)
                if True:
                    ov = pspool.tile([65, 2, 512], F32, tag="ov")
                    for c, (t0, rows) in enumerate(CHUNKS):
                        sp = pspool.tile([128, 2, 512], F32, tag="sp")
                        for bk in range(2):
                            _mm(nc, sp[:rows, bk, :QW],
                                kT[:, :, :].rearrange(
                                    "p a b -> p (a b)")[:, t0:t0 + rows],
                                qT[:, bk, :],
                                start=True, stop=False)
                            _mm(nc, sp[:rows, bk, :QW],
                                ident[:rows, :rows],
                                cmbt[:rows, c, QW * bk:QW * (bk + 1)],
                                start=False, stop=True)
                        pt = ppool.tile([128, 2, QW], BF16, tag="pt")
                        nc.scalar.activation(
                            pt[:rows, :, :], sp[:rows, :, :QW],
                            mybir.ActivationFunctionType.Exp)
                        for bk in range(2):
                            _mm(nc, ov[:, bk, :QW], vts[c][:rows, h, :],
                                pt[:rows, bk, :],
                                start=(c == 0), stop=(c == 4))
                    # stack the denominator first (the reciprocal waits
                    # on it), then copy the head output out of PSUM
                    stA, stB = st['stA'], st['stB']
                    recA, recB = st['recA'], st['recB']
                    if h >= 4:
                        nc.vector.tensor_copy(
                            stB[32 * (h - 4):32 * (h - 4) + 1, :, :],
                            ov[64:65, :, :QW])
                    oc = spool.tile([64, 2, QW], BF16, tag="och", bufs=4,
                                    name=f"och{h}_{b}")
                    nc.vector.tensor_copy(oc[:, :, :], ov[:64, :, :QW])
                    och.append(oc)
                    if h < 4:
                        nc.vector.tensor_copy(stA[32 * h:32 * h + 1, :, :],
                                              ov[64:65, :, :QW])
                        if h == 3:
                            with nc.allow_low_precision(reason="softmax den"):
                                nc.vector.reciprocal(recA[:, :, :],
                                                     stA[:, :, :])
                            rcs = [div_prep(st, hh, recA, 32 * hh)
                                   for hh in range(4)]
                            for hh in range(4):
                                division(st, hh, rcs[hh])
                    elif h == 5 and last_direct:
                        # final head of the kernel: reciprocal straight from
                        # PSUM (no stacking) to shorten the tail chain
                        rc5 = spool.tile([1, 2, QW], BF16, tag="rc5")
                        with nc.allow_low_precision(reason="softmax den"):
                            nc.vector.reciprocal(rc5[:, :, :],
                                                 ov[64:65, :, :QW])
                        division(st, 5, rc5, 0)
                    else:
                        if h == 4 and last_direct:
                            with nc.allow_low_precision(reason="softmax den"):
                                nc.vector.reciprocal(recB[:33, :, :],
                                                     stB[:33, :, :])
                            division(st, 4, recB, 0)
                        elif h == 5:
                            with nc.allow_low_precision(reason="softmax den"):
                                nc.vector.reciprocal(recB[:, :, :],
                                                     stB[:, :, :])
                            rc5 = div_prep(st, 5, recB, 32)
                            division(st, 4, recB)
                            division(st, 5, rc5)

            def outproj(b, x2t):
                for si, (t0, tn) in enumerate(TSL):
                    ps = pspool.tile([128, 2, 512], F32, tag="sp")
                    for ci in range(3):
                        _mm(nc, ps[:tn, 0, :C],
                            x2t[ci][:, :, :].rearrange(
                                "p a b -> p (a b)")[:, t0:t0 + tn],
                            wproj[:, ci, :], start=(ci == 0), stop=(ci == 2))
                    yt = spool.tile([128, C], BF16, tag="yt", bufs=3,
                                    name=f"yt{si}_{b}")
                    if nobias:
                        nc.scalar.copy(yt[:tn, :], ps[:tn, 0, :C])
                    else:
                        nc.vector.tensor_tensor(yt[:tn, :], ps[:tn, 0, :C],
                                                cb[:tn, 6 + C:6 + 2 * C],
                                                mybir.AluOpType.add)
                    (nc.sync if si % 2 == 0 else nc.gpsimd).dma_start(
                        out_d[b, t0:t0 + tn, :], yt[:tn, :])

            qkt0, vts0 = proj(0)
            qkt1, vts1 = proj(1)
            st0 = attn_init(0, qkt0, vts0)
            st1 = attn_init(1, qkt1, vts1)
            for h in range(H):
                attn_head(st0, h, last_direct=False)
            for h in range(H):
                attn_head(st1, h, last_direct=False)
            outproj(0, st0['x2t'])
            outproj(1, st1['x2t'])

    nc.compile()
    return nc


def prep_inputs(x, qkv_w, qkv_b, proj_w, proj_b, rel_pos, rel_pos_index,
                mask, patch_attn):
    x = np.asarray(x, dtype=np.float32)
    qkv_w = np.asarray(qkv_w, dtype=np.float32)
    qkv_b = np.asarray(qkv_b, dtype=np.float32)
    proj_w = np.asarray(proj_w, dtype=np.float32)
    proj_b = np.asarray(proj_b, dtype=np.float32)
    rel_pos = np.asarray(rel_pos, dtype=np.float32)
    mask = np.asarray(mask)

    # x^T padded to 580 query columns, [B, 3, 128, 580] -> [B,128,3,580]
    xT = np.zeros((B, C, NQ), dtype=np.float32)
    xT[:, :, :N] = x.transpose(0, 2, 1)
    xT = np.ascontiguousarray(xT.reshape(B, 3, 128, NQ)).astype(
        ml_dtypes.bfloat16)

    W = qkv_w.copy()
    W[:C] *= np.float32(SCALE)
    b2 = qkv_b.copy()
    b2[:C] *= np.float32(SCALE)
    # wqkv: [C, 3C]^T packed [128, 3, 3C]
    wT3 = W.T.reshape(3, 128, 3 * C)
    wqkvqk = np.ascontiguousarray(wT3[:, :, :2 * C]).astype(ml_dtypes.bfloat16)
    wqkvv = np.ascontiguousarray(wT3[:, :, 2 * C:]).astype(ml_dtypes.bfloat16)
    wproj = np.ascontiguousarray(
        proj_w.T.reshape(3, 128, C).transpose(1, 0, 2)
    ).astype(ml_dtypes.bfloat16)
    cblob = np.zeros((128, 6 + 2 * C), dtype=np.float32)
    cblob[:, :6] = b2[:2 * C].reshape(6, 128).T
    cblob[:, 6:6 + C] = np.broadcast_to(b2[2 * C:], (128, C))
    cblob[:, 6 + C:] = np.broadcast_to(proj_b, (128, C))

    # combined additive table: mask (0 / -1e30, per batch) + rel-pos bias
    # (per head), in [key, chunk, query] layout
    mb = np.zeros((B, N + 1, NQ), dtype=np.float32)
    mb[:, :N, :N] = ((mask.transpose(0, 2, 1).astype(np.float32) - 1.0)
                     * np.float32(1e30))
    if patch_attn:
        # biasT[h, key, query] over full token indices (0 = CLS, no bias)
        biasT = np.zeros((H, N, NQ), dtype=np.float32)
        rpi = np.asarray(rel_pos_index)
        biasT[:, 1:N, 1:N] = rel_pos[:, rpi].transpose(0, 2, 1)
    cmb = np.zeros((B, H, 121, 5, NQ), dtype=ml_dtypes.bfloat16)
    for b in range(B):
        for c, (t0, rows) in enumerate(CHUNKS):
            blk = mb[b, None, t0:t0 + rows, :]
            if patch_attn:
                blk = blk + biasT[:, t0:t0 + rows, :]
            cmb[b, :, :rows, c, :] = blk.astype(ml_dtypes.bfloat16)

    shared = {
        "wqkvqk": wqkvqk, "wqkvv": wqkvv, "wproj": wproj,
        "cblob": cblob,
    }
    in_maps = []
    for i in range(NCORES):
        m = dict(shared)
        m["xT"] = np.ascontiguousarray(xT[NB * i:NB * (i + 1)])
        m["cmb"] = np.ascontiguousarray(cmb[NB * i:NB * (i + 1)])
        in_maps.append(m)
    return in_maps


_NC_CACHE = {}


def _get_nc(patch_attn: bool, nobias: bool):
    key = (bool(patch_attn), bool(nobias))
    if key not in _NC_CACHE:
        _NC_CACHE[key] = build_program(*key)
    return _NC_CACHE[key]


def kernel(**inputs):
    patch_attn = bool(np.asarray(inputs["patch_attn"]))
    nobias = (not np.any(np.asarray(inputs["qkv_b"]))
              and not np.any(np.asarray(inputs["proj_b"])))
    nc = _get_nc(patch_attn, nobias)
    in_maps = prep_inputs(**inputs)
    res = bass_utils.run_bass_kernel_spmd(nc, in_maps,
                                          core_ids=list(range(NCORES)))
    out = np.concatenate([res.results[i]["out"] for i in range(NCORES)],
                         axis=0)
    return np.ascontiguousarray(out.astype(np.float32))
